# revision 8
# baseline (speedup 1.0000x reference)
"""Trainium2 Bass kernel for batched global-sum attention (B=8, C=256, N=2048).

Math (per sample b, one NeuronCore each — batch is sharded across 8 cores):
    q = Wq x + bq 1^T ; k = Wk x + bk 1^T ; v = Wv x + bv 1^T        (x: [C,N])
    qk = q^T k ;  attn = v (qk / S) ,  S = sum_b sum(qk_b)

Matmul associativity collapses the two [N,N]-sized products:
    v (q^T k) = (v q^T) k = A x + c 1^T,  A = M Wk,  M = v q^T
    A^T = (Wk^T Wq) U + u1 (x) v0sum + h (x) bv
      with U = G WvT,  G = x x^T,  u1 = Wk^T bq,  v0sum = Wv sx,  sx = x 1,
           h = (Wk^T Wq) sx + N u1
    c   = U^T (Wq^T bk) + (bq.bk) v0sum + s2 bv,  s2 = (Wq^T bk).sx + N bq.bk
    sum(qk_b) = (sum_n q).(sum_m k)

The DEVICE does the O(C^2 N) work in bf16 (fp32 PSUM accumulation): G = x x^T
from a host-pretransposed xT pack, U = G WvT, AT = QK^T U, c0 = wqb^T U, and
the output matmul attn0 = AT^T x, streamed back as bf16.
The HOST epilogue applies the exact rank-1/rank-2 bias corrections (all are
O(C N) or O(C^2), ~0.1% of the FLOPs, computed in float64 from sx) and the
global 1/S (which couples all samples and already lives in the gather step).
bf16 end-to-end keeps total HBM traffic at ~3.3 MB/core (2.26 in + 1.06 out)
vs 6.8 MB for the fp32r version; predicted rel err ~5e-3 vs the 2e-2 gate.
"""
import sys
sys.path.insert(0, '/opt/trn_rl_repo')
from contextlib import ExitStack

import numpy as np
import ml_dtypes

import concourse.bass as bass
from concourse import bacc
import concourse.mybir as mybir
import concourse.tile as tile
from concourse.bass_utils import run_bass_kernel_spmd

dt = mybir.dt
B, C, N = 8, 256, 2048
NB = N // 128
F32 = dt.float32
BF16 = dt.bfloat16
BF = ml_dtypes.bfloat16

_NC_CACHE = {}


Ident = mybir.ActivationFunctionType.Identity


def _build(warmup_mms=12, warm_n=512):
    nc = bacc.Bacc("TRN2", target_bir_lowering=False, debug=False)

    xT = nc.declare_dram_parameter("xT", [128, 4096], BF16, isOutput=False)
    x_r = nc.declare_dram_parameter("x_r", [C, N], BF16, isOutput=False)
    w2 = nc.declare_dram_parameter("w2", [C, 520], BF16, isOutput=False)
    attn = nc.declare_dram_parameter("attn", [C, N], BF16, isOutput=True)
    c_out = nc.declare_dram_parameter("c_out", [1, C], F32, isOutput=True)

    with tile.TileContext(nc) as tc, ExitStack() as ctx:
        consts = ctx.enter_context(tc.tile_pool(name="consts", bufs=1))
        xpool = ctx.enter_context(tc.tile_pool(name="xpool", bufs=1))
        small = ctx.enter_context(tc.tile_pool(name="small", bufs=1))
        outp = ctx.enter_context(tc.tile_pool(name="outp", bufs=4))
        ps_big = ctx.enter_context(tc.tile_pool(name="ps_big", bufs=4, space="PSUM"))
        ps_g = ctx.enter_context(tc.tile_pool(name="ps_g", bufs=1, space="PSUM"))
        ps_sm = ctx.enter_context(tc.tile_pool(name="ps_sm", bufs=2, space="PSUM"))

        # Input DMAs. Ring order matters (HWDGE rings are FIFO): xT chunks
        # first (they gate G, the head of the dependency chain), w2 mid
        # (needed by U at ~G-done time), x_r fine-grained last (its chunks
        # gate the attn matmuls; small chunks pipeline the ~1-2us HBM
        # completion-receipt lag on the DMA semaphores).
        t_w2 = [consts.tile([128, 520], BF16, name=f"w2_{i}") for i in range(2)]
        t_xT = xpool.tile([128, 4096], BF16, name="xTp")
        t_x = [xpool.tile([128, N], BF16, name=f"x{i}") for i in range(2)]

        # xT: 4 chunks of [128,1024] (256 KB each), 2 per ring
        nc.sync.dma_start(t_xT[:, 0:1024], xT[:, 0:1024])
        nc.scalar.dma_start(t_xT[:, 1024:2048], xT[:, 1024:2048])
        nc.sync.dma_start(t_xT[:, 2048:3072], xT[:, 2048:3072])
        nc.scalar.dma_start(t_xT[:, 3072:4096], xT[:, 3072:4096])
        nc.sync.dma_start(t_w2[0][:], w2[0:128, :])
        nc.scalar.dma_start(t_w2[1][:], w2[128:256, :])
        # x_r: per ch half, 4 column chunks of [128,512] (128 KB each)
        for mc in range(4):
            msl = slice(mc * 512, (mc + 1) * 512)
            nc.sync.dma_start(t_x[0][:, msl], x_r[0:128, msl])
            nc.scalar.dma_start(t_x[1][:, msl], x_r[128:256, msl])

        # PE warmup (no DMA dependency): wake the HAM clock gate while the
        # first input chunks are in flight.
        t_warm = consts.tile([128, warm_n], BF16, name="warm")
        nc.vector.memset(t_warm[:], 0.5)
        for wi in range(warmup_mms):
            pw = ps_big.tile([128, warm_n], F32, name="warm_ps", tag="big")
            nc.tensor.matmul(pw[:], t_warm[:, 0:128], t_warm[:],
                             start=True, stop=True)

        t_wvT = [t_w2[i][:, 0:256] for i in range(2)]
        t_QK = [t_w2[i][:, 256:512] for i in range(2)]
        t_wqb = [t_w2[i][:, 512:514] for i in range(2)]

        # G = x x^T from the transposed pack.
        t_G_ps = [ps_g.tile([128, C], F32, name=f"G{ch}", tag=f"G{ch}")
                  for ch in range(2)]
        for nb in range(NB):
            xt = t_xT[:, nb * 256:(nb + 1) * 256]
            for ch in range(2):
                nc.tensor.matmul(t_G_ps[ch][:], xt[:, ch * 128:(ch + 1) * 128],
                                 xt, start=(nb == 0), stop=(nb == NB - 1))
        t_G = [small.tile([128, C], BF16, name=f"G{ch}") for ch in range(2)]
        nc.vector.tensor_copy(t_G[0][:], t_G_ps[0][:])
        nc.scalar.activation(t_G[1][:], t_G_ps[1][:], Ident)

        # U = G WvT (G symmetric, so lhsT=G works directly)
        t_U_ps = [ps_sm.tile([128, C], F32, name=f"U_ps{ch}", tag="sm")
                  for ch in range(2)]
        for ch in range(2):
            for kt in range(2):
                nc.tensor.matmul(t_U_ps[ch][:],
                                 t_G[kt][:, ch * 128:(ch + 1) * 128],
                                 t_wvT[kt], start=(kt == 0), stop=(kt == 1))
        t_U = [small.tile([128, C], BF16, name=f"U{ch}") for ch in range(2)]
        nc.vector.tensor_copy(t_U[0][:], t_U_ps[0][:])
        nc.scalar.activation(t_U[1][:], t_U_ps[1][:], Ident)

        # AT = QK^T U ; c_row = wqb^T U
        t_AT_ps = [ps_sm.tile([128, C], F32, name=f"AT_ps{ch}", tag="sm")
                   for ch in range(2)]
        t_AT = [small.tile([128, C], BF16, name=f"AT{ch}") for ch in range(2)]
        for ch in range(2):
            csl = slice(ch * 128, (ch + 1) * 128)
            for kt in range(2):
                nc.tensor.matmul(t_AT_ps[ch][:], t_QK[kt][:, csl], t_U[kt][:],
                                 start=(kt == 0), stop=(kt == 1))
            if ch == 0:
                nc.vector.tensor_copy(t_AT[ch][:], t_AT_ps[ch][:])
            else:
                nc.scalar.activation(t_AT[ch][:], t_AT_ps[ch][:], Ident)

        t_c_ps = ps_sm.tile([2, C], F32, name="c_ps", tag="sm")
        for kt in range(2):
            nc.tensor.matmul(t_c_ps[:], t_wqb[kt], t_U[kt][:],
                             start=(kt == 0), stop=(kt == 1))
        t_c0 = small.tile([1, C], F32, name="c0")
        nc.vector.tensor_copy(t_c0[:], t_c_ps[0:1, :])
        nc.sync.dma_start(c_out[:], t_c0[:])

        # attn = AT^T x, streamed out per [128,512] chunk as bf16
        for mc in range(4):
            msl = slice(mc * 512, (mc + 1) * 512)
            for ch in range(2):
                chsl = slice(ch * 128, (ch + 1) * 128)
                pa = ps_big.tile([128, 512], F32, name=f"attn_ps{mc}_{ch}",
                                 tag="big")
                for kt in range(2):
                    nc.tensor.matmul(pa[:], t_AT[kt][:, chsl], t_x[kt][:, msl],
                                     start=(kt == 0), stop=(kt == 1))
                ta = outp.tile([128, 512], BF16, name=f"attn_sb{mc}_{ch}",
                               tag="attn_sb")
                if ch == 0:
                    nc.vector.tensor_copy(ta[:], pa[:])
                    nc.sync.dma_start(attn[chsl, msl], ta[:])
                else:
                    nc.scalar.activation(ta[:], pa[:], Ident)
                    nc.scalar.dma_start(attn[chsl, msl], ta[:])

    nc.finalize()
    return nc


def _get_nc():
    if "nc" not in _NC_CACHE:
        _NC_CACHE["nc"] = _build()
    return _NC_CACHE["nc"]


def kernel(x, Wq, bq, Wk, bk, Wv, bv):
    x = np.ascontiguousarray(x, np.float32)
    Wq = np.ascontiguousarray(Wq, np.float32)
    Wk = np.ascontiguousarray(Wk, np.float32)
    Wv = np.ascontiguousarray(Wv, np.float32)
    bq = np.ascontiguousarray(bq, np.float32)
    bk = np.ascontiguousarray(bk, np.float32)
    bv = np.ascontiguousarray(bv, np.float32)
    assert x.shape == (B, C, N), x.shape

    nc = _get_nc()

    Wq64, Wk64, Wv64 = (w.astype(np.float64) for w in (Wq, Wk, Wv))
    bq64, bk64, bv64 = (v.astype(np.float64) for v in (bq, bk, bv))
    QK = Wq64.T @ Wk64                # = (Wk^T Wq)^T
    u1 = Wk64.T @ bq64
    wqb = Wq64.T @ bk64
    bqbk = float(bq64 @ bk64)
    w2 = np.zeros((C, 520), np.float32)
    w2[:, 0:256] = Wv.T
    w2[:, 256:512] = QK.astype(np.float32)
    w2[:, 512] = wqb.astype(np.float32)
    w2 = w2.astype(BF)

    ins = []
    host = []
    for b in range(B):
        x64 = x[b].astype(np.float64)
        sx64 = x64.sum(axis=1)
        qsum = Wq64 @ sx64 + N * bq64
        ksum = Wk64 @ sx64 + N * bk64
        v0sum = Wv64 @ sx64
        h = QK.T @ sx64 + N * u1
        s2 = float(wqb @ sx64) + N * bqbk
        crest = bqbk * v0sum + s2 * bv64
        r1 = u1 @ x64
        r2 = h @ x64
        host.append((float(qsum @ ksum), v0sum, crest, r1, r2))
        xr = x[b].astype(BF)
        xTp = np.ascontiguousarray(
            xr.T.reshape(16, 128, 256).transpose(1, 0, 2).reshape(128, 4096))
        ins.append(dict(x_r=xr, xT=xTp, w2=w2))

    res = run_bass_kernel_spmd(nc, ins, list(range(B)))

    S = np.float64(np.sum([hh[0] for hh in host]))
    outs = []
    for b in range(B):
        _, v0sum, crest, r1, r2 = host[b]
        a0 = res.results[b]["attn"].astype(np.float64)
        c_full = res.results[b]["c_out"][0].astype(np.float64) + crest
        full = a0 + np.outer(v0sum, r1) + np.outer(bv64, r2) + c_full[:, None]
        outs.append((full / S).astype(np.float32))
    return np.stack(outs)


if __name__ == "__main__":
    rng = np.random.default_rng(0)
    s = 1.0 / np.sqrt(C)
    inputs = {
        "x": rng.standard_normal((B, C, N), dtype=np.float32),
        "Wq": (rng.standard_normal((C, C)) * s).astype(np.float32),
        "bq": (rng.standard_normal(C) * s).astype(np.float32),
        "Wk": (rng.standard_normal((C, C)) * s).astype(np.float32),
        "bk": (rng.standard_normal(C) * s).astype(np.float32),
        "Wv": (rng.standard_normal((C, C)) * s).astype(np.float32),
        "bv": (rng.standard_normal(C) * s).astype(np.float32),
    }
    out = kernel(**inputs)
    print("kernel output:", out.shape, out.dtype, float(np.abs(out).max()))


# revision 16
# speedup vs baseline: 1.2833x; 1.2833x over previous
"""Trainium2 Bass kernel for batched global-sum attention (B=8, C=256, N=2048).

Math (per sample b, one NeuronCore each — batch is sharded across 8 cores):
    q = Wq x + bq 1^T ; k = Wk x + bk 1^T ; v = Wv x + bv 1^T        (x: [C,N])
    qk = q^T k ;  attn = v (qk / S) ,  S = sum_b sum(qk_b)

Matmul associativity collapses the two [N,N]-sized products:
    v (q^T k) = (v q^T) k = A x + c 1^T,  A = M Wk,  M = v q^T
    A^T = (Wk^T Wq) U + u1 (x) v0sum + h (x) bv
      with U = G WvT,  G = x x^T,  u1 = Wk^T bq,  v0sum = Wv sx,  sx = x 1,
           h = (Wk^T Wq) sx + N u1
    c   = U^T (Wq^T bk) + (bq.bk) v0sum + s2 bv,  s2 = (Wq^T bk).sx + N bq.bk
    sum(qk_b) = (sum_n q).(sum_m k)

The DEVICE does the O(C^2 N) work in bf16 (fp32 PSUM accumulation): G = x x^T
from a host-pretransposed xT pack, U = G WvT, AT = QK^T U, c0 = wqb^T U, and
the output matmul attn0 = AT^T x, streamed back as bf16.
The HOST epilogue applies the exact rank-1/rank-2 bias corrections (all are
O(C N) or O(C^2), ~0.1% of the FLOPs, computed in float64 from sx) and the
global 1/S (which couples all samples and already lives in the gather step).
bf16 end-to-end keeps total HBM traffic at ~3.3 MB/core (2.26 in + 1.06 out)
vs 6.8 MB for the fp32r version; predicted rel err ~5e-3 vs the 2e-2 gate.
"""
import sys
sys.path.insert(0, '/opt/trn_rl_repo')
from contextlib import ExitStack

import numpy as np
import ml_dtypes

import concourse.bass as bass
from concourse import bacc
import concourse.mybir as mybir
import concourse.tile as tile
from concourse.bass_utils import run_bass_kernel_spmd

dt = mybir.dt
B, C, N = 8, 256, 2048
NB = N // 128
F32 = dt.float32
BF16 = dt.bfloat16
BF = ml_dtypes.bfloat16

_NC_CACHE = {}


Ident = mybir.ActivationFunctionType.Identity


def _build(warmup_mms=9, warm_n=512):
    nc = bacc.Bacc("TRN2", target_bir_lowering=False, debug=False)

    xT = nc.declare_dram_parameter("xT", [128, 4096], BF16, isOutput=False)
    x_r = nc.declare_dram_parameter("x_r", [C, N], BF16, isOutput=False)
    w2 = nc.declare_dram_parameter("w2", [C, 520], BF16, isOutput=False)
    attn = nc.declare_dram_parameter("attn", [C, N], BF16, isOutput=True)
    c_out = nc.declare_dram_parameter("c_out", [1, C], F32, isOutput=True)

    with tile.TileContext(nc) as tc, ExitStack() as ctx:
        consts = ctx.enter_context(tc.tile_pool(name="consts", bufs=1))
        xpool = ctx.enter_context(tc.tile_pool(name="xpool", bufs=1))
        small = ctx.enter_context(tc.tile_pool(name="small", bufs=1))
        outp = ctx.enter_context(tc.tile_pool(name="outp", bufs=4))
        # PSUM budget (8 banks): big 4x[128,512] (4, warm + attn rotate)
        # + G 2x[128,256] (2) + sm 2x[128,256] (2)
        ps_big = ctx.enter_context(tc.tile_pool(name="ps_big", bufs=4, space="PSUM"))
        ps_g = ctx.enter_context(tc.tile_pool(name="ps_g", bufs=1, space="PSUM"))
        ps_sm = ctx.enter_context(tc.tile_pool(name="ps_sm", bufs=2, space="PSUM"))

        # Input DMAs. Ring order matters (HWDGE rings are FIFO): xT chunks
        # first (they gate G, the head of the dependency chain), w2 mid
        # (needed by U at ~G-done time), x_r fine-grained last (its chunks
        # gate the attn matmuls; small chunks pipeline the ~1-2us HBM
        # completion-receipt lag on the DMA semaphores).
        t_w2 = [consts.tile([128, 520], BF16, name=f"w2_{i}") for i in range(2)]
        t_xT = xpool.tile([128, 4096], BF16, name="xTp")
        t_x = [xpool.tile([128, N], BF16, name=f"x{i}") for i in range(2)]

        # xT: 4 chunks of [128,1024] (256 KB each), 2 per ring
        nc.sync.dma_start(t_xT[:, 0:1024], xT[:, 0:1024])
        nc.scalar.dma_start(t_xT[:, 1024:2048], xT[:, 1024:2048])
        nc.sync.dma_start(t_xT[:, 2048:3072], xT[:, 2048:3072])
        nc.scalar.dma_start(t_xT[:, 3072:4096], xT[:, 3072:4096])
        nc.sync.dma_start(t_w2[0][:], w2[0:128, :])
        nc.scalar.dma_start(t_w2[1][:], w2[128:256, :])
        # x_r: per ch half, 2 column chunks of [128,1024] (256 KB, 2KB lines)
        for mc in range(2):
            msl = slice(mc * 1024, (mc + 1) * 1024)
            nc.sync.dma_start(t_x[0][:, msl], x_r[0:128, msl])
            nc.scalar.dma_start(t_x[1][:, msl], x_r[128:256, msl])

        # PE warmup (no DMA dependency): wake the HAM clock gate while the
        # first input chunks are in flight. One PSUM accumulation group so
        # there are no inter-matmul WAR semaphores (back-to-back issue).
        t_warm = consts.tile([128, warm_n], BF16, name="warm")
        nc.vector.memset(t_warm[:], 0.5)
        pw = ps_big.tile([128, warm_n], F32, name="warm_ps", tag="big")
        for wi in range(warmup_mms):
            nc.tensor.matmul(pw[:], t_warm[:, 0:128], t_warm[:],
                             start=(wi == 0), stop=(wi == warmup_mms - 1))

        t_wvT = [t_w2[i][:, 0:256] for i in range(2)]
        t_QK = [t_w2[i][:, 256:512] for i in range(2)]
        t_wqb = [t_w2[i][:, 512:514] for i in range(2)]

        # G = x x^T from the transposed pack. Separate PSUM tiles per row
        # chunk: a start=True on a slice of a shared tile can clobber the
        # sibling chain's accumulation (measured: uniform ~1/16 error).
        t_G_ps = [ps_g.tile([128, C], F32, name=f"G{ch}", tag=f"G{ch}")
                  for ch in range(2)]
        for nb in range(NB):
            xt = t_xT[:, nb * 256:(nb + 1) * 256]
            for ch in range(2):
                nc.tensor.matmul(t_G_ps[ch][:], xt[:, ch * 128:(ch + 1) * 128],
                                 xt, start=(nb == 0), stop=(nb == NB - 1))
        t_G = [small.tile([128, C], BF16, name=f"G{ch}") for ch in range(2)]
        nc.vector.tensor_copy(t_G[0][:], t_G_ps[0][:])
        nc.scalar.activation(t_G[1][:], t_G_ps[1][:], Ident)

        # U = G WvT (G symmetric, so lhsT=G works directly)
        t_U_ps = [ps_sm.tile([128, C], F32, name=f"U_ps{ch}", tag="sm")
                  for ch in range(2)]
        for ch in range(2):
            for kt in range(2):
                nc.tensor.matmul(t_U_ps[ch][:],
                                 t_G[kt][:, ch * 128:(ch + 1) * 128],
                                 t_wvT[kt], start=(kt == 0), stop=(kt == 1))
        t_U = [small.tile([128, C], BF16, name=f"U{ch}") for ch in range(2)]
        nc.vector.tensor_copy(t_U[0][:], t_U_ps[0][:])
        nc.scalar.activation(t_U[1][:], t_U_ps[1][:], Ident)

        # AT = QK^T U ; c_row = wqb^T U
        t_AT_ps = [ps_sm.tile([128, C], F32, name=f"AT_ps{ch}", tag="sm")
                   for ch in range(2)]
        t_AT = [small.tile([128, C], BF16, name=f"AT{ch}") for ch in range(2)]
        for ch in range(2):
            csl = slice(ch * 128, (ch + 1) * 128)
            for kt in range(2):
                nc.tensor.matmul(t_AT_ps[ch][:], t_QK[kt][:, csl], t_U[kt][:],
                                 start=(kt == 0), stop=(kt == 1))
        nc.vector.tensor_copy(t_AT[0][:], t_AT_ps[0][:])
        nc.scalar.activation(t_AT[1][:], t_AT_ps[1][:], Ident)

        t_c_ps = ps_sm.tile([2, C], F32, name="c_ps", tag="sm")
        for kt in range(2):
            nc.tensor.matmul(t_c_ps[:], t_wqb[kt], t_U[kt][:],
                             start=(kt == 0), stop=(kt == 1))
        t_c0 = small.tile([1, C], F32, name="c0")
        nc.vector.tensor_copy(t_c0[:], t_c_ps[0:1, :])
        nc.sync.dma_start(c_out[:], t_c0[:])

        # attn = AT^T x in [128,1024] super-chunks; each PSUM chunk is
        # copied by BOTH vector (first half) and scalar (second half), then
        # DMA'd out on the sync ring (scalar engine stays copy-only here,
        # and the ring order matches chunk readiness order).
        for s in range(2):
            for ch in range(2):
                chsl = slice(ch * 128, (ch + 1) * 128)
                pa = [ps_big.tile([128, 512], F32, name=f"attn_ps{s}_{ch}_{h}",
                                  tag="big") for h in range(2)]
                for h in range(2):
                    msl = slice(s * 1024 + h * 512, s * 1024 + (h + 1) * 512)
                    for kt in range(2):
                        nc.tensor.matmul(pa[h][:], t_AT[kt][:, chsl],
                                         t_x[kt][:, msl],
                                         start=(kt == 0), stop=(kt == 1))
                ta = outp.tile([128, 1024], BF16, name=f"attn_sb{s}_{ch}",
                               tag="attn_sb")
                nc.vector.tensor_copy(ta[:, 0:512], pa[0][:])
                nc.scalar.activation(ta[:, 512:1024], pa[1][:], Ident)
                nc.sync.dma_start(attn[chsl, s * 1024:(s + 1) * 1024], ta[:])

    nc.finalize()
    return nc


def _get_nc():
    if "nc" not in _NC_CACHE:
        _NC_CACHE["nc"] = _build()
    return _NC_CACHE["nc"]


def kernel(x, Wq, bq, Wk, bk, Wv, bv):
    x = np.ascontiguousarray(x, np.float32)
    Wq = np.ascontiguousarray(Wq, np.float32)
    Wk = np.ascontiguousarray(Wk, np.float32)
    Wv = np.ascontiguousarray(Wv, np.float32)
    bq = np.ascontiguousarray(bq, np.float32)
    bk = np.ascontiguousarray(bk, np.float32)
    bv = np.ascontiguousarray(bv, np.float32)
    assert x.shape == (B, C, N), x.shape

    nc = _get_nc()

    Wq64, Wk64, Wv64 = (w.astype(np.float64) for w in (Wq, Wk, Wv))
    bq64, bk64, bv64 = (v.astype(np.float64) for v in (bq, bk, bv))
    QK = Wq64.T @ Wk64                # = (Wk^T Wq)^T
    u1 = Wk64.T @ bq64
    wqb = Wq64.T @ bk64
    bqbk = float(bq64 @ bk64)
    w2 = np.zeros((C, 520), np.float32)
    w2[:, 0:256] = Wv.T
    w2[:, 256:512] = QK.astype(np.float32)
    w2[:, 512] = wqb.astype(np.float32)
    w2 = w2.astype(BF)

    ins = []
    host = []
    for b in range(B):
        x64 = x[b].astype(np.float64)
        sx64 = x64.sum(axis=1)
        qsum = Wq64 @ sx64 + N * bq64
        ksum = Wk64 @ sx64 + N * bk64
        v0sum = Wv64 @ sx64
        h = QK.T @ sx64 + N * u1
        s2 = float(wqb @ sx64) + N * bqbk
        crest = bqbk * v0sum + s2 * bv64
        r1 = u1 @ x64
        r2 = h @ x64
        host.append((float(qsum @ ksum), v0sum, crest, r1, r2))
        xr = x[b].astype(BF)
        xTp = np.ascontiguousarray(
            xr.T.reshape(16, 128, 256).transpose(1, 0, 2).reshape(128, 4096))
        ins.append(dict(x_r=xr, xT=xTp, w2=w2))

    res = run_bass_kernel_spmd(nc, ins, list(range(B)))

    S = np.float64(np.sum([hh[0] for hh in host]))
    outs = []
    for b in range(B):
        _, v0sum, crest, r1, r2 = host[b]
        a0 = res.results[b]["attn"].astype(np.float64)
        c_full = res.results[b]["c_out"][0].astype(np.float64) + crest
        full = a0 + np.outer(v0sum, r1) + np.outer(bv64, r2) + c_full[:, None]
        outs.append((full / S).astype(np.float32))
    return np.stack(outs)


if __name__ == "__main__":
    rng = np.random.default_rng(0)
    s = 1.0 / np.sqrt(C)
    inputs = {
        "x": rng.standard_normal((B, C, N), dtype=np.float32),
        "Wq": (rng.standard_normal((C, C)) * s).astype(np.float32),
        "bq": (rng.standard_normal(C) * s).astype(np.float32),
        "Wk": (rng.standard_normal((C, C)) * s).astype(np.float32),
        "bk": (rng.standard_normal(C) * s).astype(np.float32),
        "Wv": (rng.standard_normal((C, C)) * s).astype(np.float32),
        "bv": (rng.standard_normal(C) * s).astype(np.float32),
    }
    out = kernel(**inputs)
    print("kernel output:", out.shape, out.dtype, float(np.abs(out).max()))


# revision 19
# speedup vs baseline: 1.3287x; 1.0354x over previous
"""Trainium2 Bass kernel for batched global-sum attention (B=8, C=256, N=2048).

Math (per sample b, one NeuronCore each — batch is sharded across 8 cores):
    q = Wq x + bq 1^T ; k = Wk x + bk 1^T ; v = Wv x + bv 1^T        (x: [C,N])
    qk = q^T k ;  attn = v (qk / S) ,  S = sum_b sum(qk_b)

Matmul associativity collapses the two [N,N]-sized products:
    v (q^T k) = (v q^T) k = A x + c 1^T,  A = M Wk,  M = v q^T
    A^T = (Wk^T Wq) U + u1 (x) v0sum + h (x) bv
      with U = G WvT,  G = x x^T,  u1 = Wk^T bq,  v0sum = Wv sx,  sx = x 1,
           h = (Wk^T Wq) sx + N u1
    c   = U^T (Wq^T bk) + (bq.bk) v0sum + s2 bv,  s2 = (Wq^T bk).sx + N bq.bk
    sum(qk_b) = (sum_n q).(sum_m k)

The DEVICE does the O(C^2 N) work in bf16 (fp32 PSUM accumulation): G = x x^T
from a host-pretransposed xT pack, U = G WvT, AT = QK^T U, c0 = wqb^T U, and
the output matmul attn0 = AT^T x, streamed back as bf16.
The HOST epilogue applies the exact rank-1/rank-2 bias corrections (all are
O(C N) or O(C^2), ~0.1% of the FLOPs, computed in float64 from sx) and the
global 1/S (which couples all samples and already lives in the gather step).
bf16 end-to-end keeps total HBM traffic at ~3.3 MB/core (2.26 in + 1.06 out)
vs 6.8 MB for the fp32r version; predicted rel err ~5e-3 vs the 2e-2 gate.
"""
import sys
sys.path.insert(0, '/opt/trn_rl_repo')
from contextlib import ExitStack

import numpy as np
import ml_dtypes

import concourse.bass as bass
from concourse import bacc
import concourse.mybir as mybir
import concourse.tile as tile
from concourse.bass_utils import run_bass_kernel_spmd

dt = mybir.dt
B, C, N = 8, 256, 2048
NB = N // 128
F32 = dt.float32
BF16 = dt.bfloat16
BF = ml_dtypes.bfloat16

_NC_CACHE = {}


Ident = mybir.ActivationFunctionType.Identity


def _build(warmup_mms=5, warm_n=512):
    nc = bacc.Bacc("TRN2", target_bir_lowering=False, debug=False)

    xT = nc.declare_dram_parameter("xT", [128, 4096], BF16, isOutput=False)
    x_r = nc.declare_dram_parameter("x_r", [C, N], BF16, isOutput=False)
    w2 = nc.declare_dram_parameter("w2", [C, 520], BF16, isOutput=False)
    attn = nc.declare_dram_parameter("attn", [C, N], BF16, isOutput=True)
    c_out = nc.declare_dram_parameter("c_out", [1, C], F32, isOutput=True)

    with tile.TileContext(nc) as tc, ExitStack() as ctx:
        consts = ctx.enter_context(tc.tile_pool(name="consts", bufs=1))
        xpool = ctx.enter_context(tc.tile_pool(name="xpool", bufs=1))
        small = ctx.enter_context(tc.tile_pool(name="small", bufs=1))
        outp = ctx.enter_context(tc.tile_pool(name="outp", bufs=4))
        # PSUM budget (8 banks): big 4x[128,512] (4, warm + attn rotate)
        # + G 2x[128,256] (2) + sm 2x[128,256] (2)
        ps_big = ctx.enter_context(tc.tile_pool(name="ps_big", bufs=4, space="PSUM"))
        ps_g = ctx.enter_context(tc.tile_pool(name="ps_g", bufs=1, space="PSUM"))
        ps_sm = ctx.enter_context(tc.tile_pool(name="ps_sm", bufs=2, space="PSUM"))

        # Input DMAs. Ring order matters (HWDGE rings are FIFO): xT chunks
        # first (they gate G, the head of the dependency chain), w2 mid
        # (needed by U at ~G-done time), x_r fine-grained last (its chunks
        # gate the attn matmuls; small chunks pipeline the ~1-2us HBM
        # completion-receipt lag on the DMA semaphores).
        t_w2 = [consts.tile([128, 520], BF16, name=f"w2_{i}") for i in range(2)]
        t_xT = xpool.tile([128, 4096], BF16, name="xTp")
        t_x = [xpool.tile([128, N], BF16, name=f"x{i}") for i in range(2)]

        # xT: a small 128 KB chunk first on each ring (early G start — the
        # DMA-completion receipt lag is paid on less data), then the 384 KB
        # remainder. The G loop consumes nb in matching order.
        nc.sync.dma_start(t_xT[:, 0:512], xT[:, 0:512])
        nc.scalar.dma_start(t_xT[:, 2048:2560], xT[:, 2048:2560])
        nc.sync.dma_start(t_xT[:, 512:2048], xT[:, 512:2048])
        nc.scalar.dma_start(t_xT[:, 2560:4096], xT[:, 2560:4096])
        nc.sync.dma_start(t_w2[0][:], w2[0:128, :])
        nc.scalar.dma_start(t_w2[1][:], w2[128:256, :])
        # x_r: per ch half, 2 column chunks of [128,1024] (256 KB, 2KB lines)
        for mc in range(2):
            msl = slice(mc * 1024, (mc + 1) * 1024)
            nc.sync.dma_start(t_x[0][:, msl], x_r[0:128, msl])
            nc.scalar.dma_start(t_x[1][:, msl], x_r[128:256, msl])

        # PE warmup (no DMA dependency): wake the HAM clock gate while the
        # first input chunks are in flight. One PSUM accumulation group so
        # there are no inter-matmul WAR semaphores (back-to-back issue).
        t_warm = consts.tile([128, warm_n], BF16, name="warm")
        nc.vector.memset(t_warm[:], 0.5)
        pw = ps_big.tile([128, warm_n], F32, name="warm_ps", tag="big")
        for wi in range(warmup_mms):
            nc.tensor.matmul(pw[:], t_warm[:, 0:128], t_warm[:],
                             start=(wi == 0), stop=(wi == warmup_mms - 1))

        t_wvT = [t_w2[i][:, 0:256] for i in range(2)]
        t_QK = [t_w2[i][:, 256:512] for i in range(2)]
        t_wqb = [t_w2[i][:, 512:514] for i in range(2)]

        # G = x x^T from the transposed pack. Separate PSUM tiles per row
        # chunk: a start=True on a slice of a shared tile can clobber the
        # sibling chain's accumulation (measured: uniform ~1/16 error).
        t_G_ps = [ps_g.tile([128, C], F32, name=f"G{ch}", tag=f"G{ch}")
                  for ch in range(2)]
        nb_order = [0, 1, 8, 9, 2, 3, 4, 5, 6, 7, 10, 11, 12, 13, 14, 15]
        for i, nb in enumerate(nb_order):
            xt = t_xT[:, nb * 256:(nb + 1) * 256]
            for ch in range(2):
                nc.tensor.matmul(t_G_ps[ch][:], xt[:, ch * 128:(ch + 1) * 128],
                                 xt, start=(i == 0), stop=(i == NB - 1))
        t_G = [small.tile([128, C], BF16, name=f"G{ch}") for ch in range(2)]
        nc.vector.tensor_copy(t_G[0][:], t_G_ps[0][:])
        nc.scalar.activation(t_G[1][:], t_G_ps[1][:], Ident)

        # U = G WvT (G symmetric, so lhsT=G works directly)
        t_U_ps = [ps_sm.tile([128, C], F32, name=f"U_ps{ch}", tag="sm")
                  for ch in range(2)]
        for ch in range(2):
            for kt in range(2):
                nc.tensor.matmul(t_U_ps[ch][:],
                                 t_G[kt][:, ch * 128:(ch + 1) * 128],
                                 t_wvT[kt], start=(kt == 0), stop=(kt == 1))
        t_U = [small.tile([128, C], BF16, name=f"U{ch}") for ch in range(2)]
        nc.vector.tensor_copy(t_U[0][:], t_U_ps[0][:])
        nc.scalar.activation(t_U[1][:], t_U_ps[1][:], Ident)

        # AT = QK^T U ; c_row = wqb^T U
        t_AT_ps = [ps_sm.tile([128, C], F32, name=f"AT_ps{ch}", tag="sm")
                   for ch in range(2)]
        t_AT = [small.tile([128, C], BF16, name=f"AT{ch}") for ch in range(2)]
        for ch in range(2):
            csl = slice(ch * 128, (ch + 1) * 128)
            for kt in range(2):
                nc.tensor.matmul(t_AT_ps[ch][:], t_QK[kt][:, csl], t_U[kt][:],
                                 start=(kt == 0), stop=(kt == 1))
        nc.vector.tensor_copy(t_AT[0][:], t_AT_ps[0][:])
        nc.scalar.activation(t_AT[1][:], t_AT_ps[1][:], Ident)

        t_c_ps = ps_sm.tile([2, C], F32, name="c_ps", tag="sm")
        for kt in range(2):
            nc.tensor.matmul(t_c_ps[:], t_wqb[kt], t_U[kt][:],
                             start=(kt == 0), stop=(kt == 1))
        t_c0 = small.tile([1, C], F32, name="c0")
        nc.vector.tensor_copy(t_c0[:], t_c_ps[0:1, :])
        nc.sync.dma_start(c_out[:], t_c0[:])

        # attn = AT^T x in [128,1024] super-chunks; each PSUM chunk is
        # copied by BOTH vector (first half) and scalar (second half), then
        # DMA'd out on the sync ring (scalar engine stays copy-only here,
        # and the ring order matches chunk readiness order).
        for s in range(2):
            for ch in range(2):
                chsl = slice(ch * 128, (ch + 1) * 128)
                pa = [ps_big.tile([128, 512], F32, name=f"attn_ps{s}_{ch}_{h}",
                                  tag="big") for h in range(2)]
                for h in range(2):
                    msl = slice(s * 1024 + h * 512, s * 1024 + (h + 1) * 512)
                    for kt in range(2):
                        nc.tensor.matmul(pa[h][:], t_AT[kt][:, chsl],
                                         t_x[kt][:, msl],
                                         start=(kt == 0), stop=(kt == 1))
                ta = outp.tile([128, 1024], BF16, name=f"attn_sb{s}_{ch}",
                               tag="attn_sb")
                nc.vector.tensor_copy(ta[:, 0:512], pa[0][:])
                nc.scalar.activation(ta[:, 512:1024], pa[1][:], Ident)
                nc.sync.dma_start(attn[chsl, s * 1024:(s + 1) * 1024], ta[:])

    nc.finalize()
    return nc


def _get_nc():
    if "nc" not in _NC_CACHE:
        _NC_CACHE["nc"] = _build()
    return _NC_CACHE["nc"]


def kernel(x, Wq, bq, Wk, bk, Wv, bv):
    x = np.ascontiguousarray(x, np.float32)
    Wq = np.ascontiguousarray(Wq, np.float32)
    Wk = np.ascontiguousarray(Wk, np.float32)
    Wv = np.ascontiguousarray(Wv, np.float32)
    bq = np.ascontiguousarray(bq, np.float32)
    bk = np.ascontiguousarray(bk, np.float32)
    bv = np.ascontiguousarray(bv, np.float32)
    assert x.shape == (B, C, N), x.shape

    nc = _get_nc()

    Wq64, Wk64, Wv64 = (w.astype(np.float64) for w in (Wq, Wk, Wv))
    bq64, bk64, bv64 = (v.astype(np.float64) for v in (bq, bk, bv))
    QK = Wq64.T @ Wk64                # = (Wk^T Wq)^T
    u1 = Wk64.T @ bq64
    wqb = Wq64.T @ bk64
    bqbk = float(bq64 @ bk64)
    w2 = np.zeros((C, 520), np.float32)
    w2[:, 0:256] = Wv.T
    w2[:, 256:512] = QK.astype(np.float32)
    w2[:, 512] = wqb.astype(np.float32)
    w2 = w2.astype(BF)

    ins = []
    host = []
    for b in range(B):
        x64 = x[b].astype(np.float64)
        sx64 = x64.sum(axis=1)
        qsum = Wq64 @ sx64 + N * bq64
        ksum = Wk64 @ sx64 + N * bk64
        v0sum = Wv64 @ sx64
        h = QK.T @ sx64 + N * u1
        s2 = float(wqb @ sx64) + N * bqbk
        crest = bqbk * v0sum + s2 * bv64
        r1 = u1 @ x64
        r2 = h @ x64
        host.append((float(qsum @ ksum), v0sum, crest, r1, r2))
        xr = x[b].astype(BF)
        xTp = np.ascontiguousarray(
            xr.T.reshape(16, 128, 256).transpose(1, 0, 2).reshape(128, 4096))
        ins.append(dict(x_r=xr, xT=xTp, w2=w2))

    res = run_bass_kernel_spmd(nc, ins, list(range(B)))

    S = np.float64(np.sum([hh[0] for hh in host]))
    outs = []
    for b in range(B):
        _, v0sum, crest, r1, r2 = host[b]
        a0 = res.results[b]["attn"].astype(np.float64)
        c_full = res.results[b]["c_out"][0].astype(np.float64) + crest
        full = a0 + np.outer(v0sum, r1) + np.outer(bv64, r2) + c_full[:, None]
        outs.append((full / S).astype(np.float32))
    return np.stack(outs)


if __name__ == "__main__":
    rng = np.random.default_rng(0)
    s = 1.0 / np.sqrt(C)
    inputs = {
        "x": rng.standard_normal((B, C, N), dtype=np.float32),
        "Wq": (rng.standard_normal((C, C)) * s).astype(np.float32),
        "bq": (rng.standard_normal(C) * s).astype(np.float32),
        "Wk": (rng.standard_normal((C, C)) * s).astype(np.float32),
        "bk": (rng.standard_normal(C) * s).astype(np.float32),
        "Wv": (rng.standard_normal((C, C)) * s).astype(np.float32),
        "bv": (rng.standard_normal(C) * s).astype(np.float32),
    }
    out = kernel(**inputs)
    print("kernel output:", out.shape, out.dtype, float(np.abs(out).max()))
